# revision 1
# baseline (speedup 1.0000x reference)
"""Generalized Hamiltonian Dynamics kernel — data-parallel across 8 NeuronCores.

Strategy (per sharding_hint): shard z along the batch axis (32768 -> 8 x 4096),
replicate the small MLP weights on every core. The Hamiltonian gradient is
computed with an explicit closed-form backward pass (no autodiff):

    h1 = tanh(z @ W1 + b1)
    h2 = tanh(h1 @ W2 + b2)
    dH/dh2 = W3^T          (H = sum(h2 @ W3 + b3))
    g2 = (1 - h2^2) * W3^T
    g1 = (1 - h1^2) * (g2 @ W2^T)
    gradH = g1 @ W1^T
    out = concat(gradH[:, 32:], -gradH[:, :32]) + tanh(z @ Wf1 + bf1) @ Wf2 + bf2
"""

import numpy as np
import jax
import jax.numpy as jnp
from functools import partial

BATCH, DIN, HID = 32768, 64, 1024
N_CORES = 8


def _ghd_shard(z, W1, b1, W2, b2, W3, b3, Wf1, bf1, Wf2, bf2):
    # z: [B/8, 64]; weights replicated.
    h1_pre = z @ W1 + b1
    h1 = jnp.tanh(h1_pre)
    h2_pre = h1 @ W2 + b2
    h2 = jnp.tanh(h2_pre)
    # Backward through H = sum(h2 @ W3 + b3): dH/dh2 rows are all W3^T.
    w3row = W3[:, 0]                      # [HID]
    g2 = (1.0 - h2 * h2) * w3row          # [B/8, HID]
    g1 = (1.0 - h1 * h1) * (g2 @ W2.T)    # [B/8, HID]
    gradH = g1 @ W1.T                     # [B/8, DIN]
    dim = DIN // 2
    hnn = jnp.concatenate([gradH[:, dim:], -gradH[:, :dim]], axis=-1)
    forcing = jnp.tanh(z @ Wf1 + bf1) @ Wf2 + bf2
    return hnn + forcing


_pmapped = None


def _get_pmapped(n):
    global _pmapped
    if _pmapped is None:
        _pmapped = jax.pmap(
            _ghd_shard,
            in_axes=(0,) + (None,) * 10,
            devices=jax.devices()[:n],
        )
    return _pmapped


def kernel(z, W1, b1, W2, b2, W3, b3, Wf1, bf1, Wf2, bf2):
    z = np.asarray(z, dtype=np.float32)
    args = [np.asarray(a, dtype=np.float32)
            for a in (W1, b1, W2, b2, W3, b3, Wf1, bf1, Wf2, bf2)]
    n = min(N_CORES, jax.device_count())
    b = z.shape[0]
    if b % n != 0:
        n = 1
    zs = z.reshape(n, b // n, z.shape[1])
    fn = _get_pmapped(n)
    out = fn(zs, *args)                   # [n, B/n, DIN]
    return np.asarray(out).reshape(b, z.shape[1]).astype(np.float32)



# revision 7
# speedup vs baseline: 3.0400x; 3.0400x over previous
"""Generalized Hamiltonian Dynamics — Bass/Tile kernel, data-parallel on 8 NeuronCores.

Strategy (per sharding_hint): shard z along batch (32768 -> 8 x 4096), replicate
the small MLP weights. Per core, feature-major compute in fp16 with fp32 PSUM:

    zT   = PE-transpose(z)                                [64, n]   (dup to 128p)
    h1f  = tanh(W1fT zT + b1f)     W1f=[W1|Wf1]           [2048, n] (16 m-tiles)
    sq1  = h1^2                                           [1024, n]
    h2   = tanh(W2T h1 + b2)                              [1024, n]
    g2n  = (h2^2 - 1) * w3row      (= -g2)                [1024, n]
    un   = W2 g2n                  (= -u)                 [1024, n]
    g1   = (sq1 - 1) * un          (= (1-h1^2)*u)         [1024, n]
    outT = [PW1;Wf2]^T [g1;f1] + bf2                      [64, n]
    out  = PE-transpose(outT)                             [n, 64]

where PW1 = [W1[32:64]; -W1[0:32]] folds the symplectic permutation into the
output projection. L1 (K=64) runs row-packed: two concurrent matmuls in PE
row-groups 0-1 / 2-3 via base_partition 0/64.
"""

import os
import tempfile
import threading

import numpy as np

BATCH, DIN, HID = 32768, 64, 1024
NCORE = 8
BC = BATCH // NCORE      # 4096 rows per core
NT = 512                 # batch tile (matmul free dim)
MT = HID // 128          # 8 m-tiles per hidden layer

_lock = threading.Lock()
_state = {}


# ---------------------------------------------------------------- program ----
def _build_program(n_rows=BC):
    from contextlib import ExitStack

    import concourse.bacc as bacc
    import concourse.mybir as mybir
    import concourse.tile as tile
    from concourse.bass import ts

    F16 = mybir.dt.float16
    F32 = mybir.dt.float32
    TANH = mybir.ActivationFunctionType.Tanh
    IDENT = mybir.ActivationFunctionType.Identity
    MUL = mybir.AluOpType.mult
    SUB = mybir.AluOpType.subtract

    nnt = n_rows // NT

    nc = bacc.Bacc(
        "TRN2", target_bir_lowering=False, debug=False, num_devices=NCORE
    )
    z_d = nc.dram_tensor("z", [n_rows, DIN], F16, kind="ExternalInput")
    w1f_d = nc.dram_tensor("w1f", [128, 8 * 128], F16, kind="ExternalInput")
    w2_d = nc.dram_tensor("w2", [128, 64 * 128], F16, kind="ExternalInput")
    w2t_d = nc.dram_tensor("w2t", [128, 64 * 128], F16, kind="ExternalInput")
    wo_d = nc.dram_tensor("wout", [128, 16 * 64], F16, kind="ExternalInput")
    # bias cols: 0:16 b1f | 16:24 b2 | 24:32 w3row | 32 bf2(pad128)
    bias_d = nc.dram_tensor("bias", [128, 33], F32, kind="ExternalInput")
    id_d = nc.dram_tensor("ident", [128, 128], F16, kind="ExternalInput")
    out_d = nc.dram_tensor("out", [n_rows, DIN], F16, kind="ExternalOutput")

    with tile.TileContext(nc) as tc, ExitStack() as ctx:
        wp = ctx.enter_context(tc.tile_pool(name="weights", bufs=1))
        zp = ctx.enter_context(tc.tile_pool(name="zin", bufs=2))
        ztp = ctx.enter_context(tc.tile_pool(name="zt", bufs=2))
        h1p = ctx.enter_context(tc.tile_pool(name="h1f", bufs=2))
        sqp = ctx.enter_context(tc.tile_pool(name="sq1", bufs=2))
        g2p = ctx.enter_context(tc.tile_pool(name="g2", bufs=2))
        g1p = ctx.enter_context(tc.tile_pool(name="g1", bufs=2))
        op = ctx.enter_context(tc.tile_pool(name="osb", bufs=2))
        scr = ctx.enter_context(tc.tile_pool(name="scr", bufs=2))
        ps = ctx.enter_context(tc.tile_pool(name="ps", bufs=1, space="PSUM"))

        w1f_s = wp.tile_from(w1f_d.ap())
        w2_s = wp.tile_from(w2_d.ap())
        w2t_s = wp.tile_from(w2t_d.ap())
        wo_s = wp.tile_from(wo_d.ap())
        bias_s = wp.tile_from(bias_d.ap())
        id_s = wp.tile_from(id_d.ap())

        z_ap = z_d.ap()
        out_ap = out_d.ap()

        def w2blk(w, k, m):
            return w[:, (k * 8 + m) * 128:(k * 8 + m + 1) * 128]

        def stage_TA(nt):
            """z load + PE transpose + dup, then L1+F1 row-packed pairs."""
            zraw = zp.tile([128, 4, 64], F16, tag="zraw")
            src = z_ap[nt * NT:(nt + 1) * NT, :].rearrange(
                "(c p) j -> p c j", p=128
            )
            nc.sync.dma_start(out=zraw[:], in_=src)
            zps = ps.tile([64, NT], F16, tag="psZ", bufs=1)
            for c in range(4):
                nc.tensor.transpose(
                    zps[:, ts(c, 128)], zraw[:, c, :], id_s[:, :]
                )
            zt = ztp.tile([128, NT], F16, tag="zt")
            nc.scalar.copy(zt[0:64, :], zps[:, :])
            nc.vector.tensor_copy(zt[64:128, :], zps[:, :])

            h1f = h1p.tile([128, 16, NT], F16, tag="h1f")
            sq1 = sqp.tile([128, 8, NT], F16, tag="sq1")
            for p in range(8):
                pa = ps.tile([128, NT], F32, tag="psA", bufs=3)
                pb = ps.tile([128, NT], F32, tag="psA", bufs=3)
                nc.tensor.matmul(
                    pa[:], lhsT=w1f_s[0:64, ts(p, 128)], rhs=zt[0:64, :],
                    start=True, stop=True,
                )
                nc.tensor.matmul(
                    pb[:], lhsT=w1f_s[64:128, ts(p, 128)], rhs=zt[64:128, :],
                    start=True, stop=True,
                )
                for m, pp in ((2 * p, pa), (2 * p + 1, pb)):
                    nc.scalar.activation(
                        h1f[:, m, :], pp[:], TANH, bias=bias_s[:, m:m + 1]
                    )
                    if m < 8:
                        nc.vector.tensor_tensor(
                            sq1[:, m, :], h1f[:, m, :], h1f[:, m, :], MUL
                        )
            return h1f, sq1

        def stage_BCDO(nt, h1f, sq1):
            # ---- L2 forward + g2neg
            g2 = g2p.tile([128, 8, NT], F16, tag="g2")
            for m in range(8):
                pp = ps.tile([128, NT], F32, tag="psBC", bufs=2)
                for k in range(8):
                    nc.tensor.matmul(
                        pp[:], lhsT=w2blk(w2_s, k, m), rhs=h1f[:, k, :],
                        start=(k == 0), stop=(k == 7),
                    )
                h2 = scr.tile([128, NT], F16, tag="h2")
                nc.scalar.activation(
                    h2[:], pp[:], TANH, bias=bias_s[:, 16 + m:17 + m]
                )
                sq2 = scr.tile([128, NT], F16, tag="sq2")
                nc.vector.tensor_tensor(sq2[:], h2[:], h2[:], MUL)
                w3c = bias_s[:, 24 + m:25 + m].to_broadcast((128, NT))
                nc.vector.scalar_tensor_tensor(
                    g2[:, m, :], sq2[:], 1.0, w3c, SUB, MUL
                )
            # ---- L2 backward: un = W2 g2n ; g1 = (sq1-1)*un
            g1 = g1p.tile([128, 8, NT], F16, tag="g1")
            for m in range(8):
                pp = ps.tile([128, NT], F32, tag="psBC", bufs=2)
                for k in range(8):
                    nc.tensor.matmul(
                        pp[:], lhsT=w2blk(w2t_s, k, m), rhs=g2[:, k, :],
                        start=(k == 0), stop=(k == 7),
                    )
                nc.vector.scalar_tensor_tensor(
                    g1[:, m, :], sq1[:, m, :], 1.0, pp[:], SUB, MUL
                )
            # ---- output projection: outT = [PW1;Wf2]^T [g1;f1] + bf2
            po = ps.tile([64, NT], F32, tag="psDO", bufs=2)
            for k in range(16):
                rhs = g1[:, k, :] if k < 8 else h1f[:, k, :]
                nc.tensor.matmul(
                    po[:], lhsT=wo_s[:, ts(k, 64)], rhs=rhs,
                    start=(k == 0), stop=(k == 15),
                )
            osb = op.tile([64, NT], F16, tag="osb")
            nc.scalar.activation(
                osb[:], po[:], IDENT, bias=bias_s[0:64, 32:33]
            )
            # ---- transpose back to batch-major and store
            pot = ps.tile([128, 4 * 64], F16, tag="psDO", bufs=2)
            for j in range(4):
                nc.tensor.transpose(
                    pot[:, ts(j, 64)], osb[:, ts(j, 128)], id_s[0:64, 0:64]
                )
            ot2 = op.tile([128, 4, 64], F16, tag="ot2")
            nc.vector.tensor_copy(ot2[:], pot[:])
            dst = out_ap[nt * NT:(nt + 1) * NT, :].rearrange(
                "(c p) j -> p c j", p=128
            )
            nc.sync.dma_start(out=dst, in_=ot2[:])

        prev = None
        for nt in range(nnt):
            cur = stage_TA(nt)
            if prev is not None:
                stage_BCDO(nt - 1, *prev)
            prev = cur
        stage_BCDO(nnt - 1, *prev)

    nc.compile()
    return nc


# ------------------------------------------------------------- host packing --
def _pack_weights(W1, b1, W2, b2, W3, b3, Wf1, bf1, Wf2, bf2):
    f16 = np.float16
    W1 = np.asarray(W1, np.float32)
    W2 = np.asarray(W2, np.float32)
    W3 = np.asarray(W3, np.float32)
    Wf1 = np.asarray(Wf1, np.float32)
    Wf2 = np.asarray(Wf2, np.float32)

    W1f = np.concatenate([W1, Wf1], axis=1)            # [64, 2048]
    t = W1f.reshape(64, 8, 2, 128)
    w1f = np.concatenate([t[:, :, 0, :], t[:, :, 1, :]], axis=0)  # [128,8,128]
    w1f = np.ascontiguousarray(w1f.reshape(128, 1024)).astype(f16)

    def pack_kxm(W):                                    # [1024,1024] blocks
        return np.ascontiguousarray(
            W.reshape(8, 128, 8, 128).transpose(1, 0, 2, 3).reshape(128, 8192)
        ).astype(f16)

    w2 = pack_kxm(W2)
    w2t = pack_kxm(np.ascontiguousarray(W2.T))

    # gradT = W1 @ g1T with W1 [DIN=64, HID]; hnnT[r] = gradT[32+r] (r<32)
    # else -gradT[r-32] -> permute/sign W1 rows.
    PW1 = np.concatenate([W1[32:64, :], -W1[0:32, :]], axis=0)  # [64, 1024]
    lhs = np.concatenate(
        [PW1.T.reshape(8, 128, 64), Wf2.reshape(8, 128, 64)], axis=0
    )                                                   # [16,128,64]
    wout = np.ascontiguousarray(
        lhs.transpose(1, 0, 2).reshape(128, 1024)
    ).astype(f16)

    bias = np.zeros((128, 33), np.float32)
    bias[:, 0:16] = np.concatenate(
        [np.asarray(b1, np.float32), np.asarray(bf1, np.float32)]
    ).reshape(16, 128).T
    bias[:, 16:24] = np.asarray(b2, np.float32).reshape(8, 128).T
    bias[:, 24:32] = W3[:, 0].reshape(8, 128).T
    bias[0:64, 32] = np.asarray(bf2, np.float32)

    ident = np.eye(128, dtype=f16)
    return {"w1f": w1f, "w2": w2, "w2t": w2t, "wout": wout,
            "bias": bias, "ident": ident}


def _weights_key(*arrs):
    import hashlib

    h = hashlib.blake2b(digest_size=16)
    for a in arrs:
        a = np.asarray(a)
        h.update(a.shape.__repr__().encode())
        h.update(np.ascontiguousarray(a).view(np.uint8).data)
    return h.hexdigest()


# ----------------------------------------------------------------- runners ---
def _axon_runner(nc):
    """Build a cached jitted executor for the Bass module via PJRT/shard_map."""
    import jax
    import jax.numpy as jnp
    from jax.experimental.shard_map import shard_map
    from jax.sharding import Mesh, NamedSharding, PartitionSpec

    import concourse.mybir as mybir
    from concourse import bass2jax

    bass2jax.install_neuronx_cc_hook()

    partition_name = (
        nc.partition_id_tensor.name if nc.partition_id_tensor else None
    )
    in_names, out_names, out_avals = [], [], []
    for alloc in nc.m.functions[0].allocations:
        if not isinstance(alloc, mybir.MemoryLocationSet):
            continue
        name = alloc.memorylocations[0].name
        if alloc.kind == "ExternalInput":
            if name != partition_name:
                in_names.append(name)
        elif alloc.kind == "ExternalOutput":
            out_names.append(name)
            out_avals.append(
                jax.core.ShapedArray(
                    tuple(alloc.tensor_shape), mybir.dt.np(alloc.dtype)
                )
            )
    n_params = len(in_names)
    all_names = in_names + out_names
    if partition_name is not None:
        all_names.append(partition_name)

    def _body(*args):
        operands = list(args)
        if partition_name is not None:
            operands.append(bass2jax.partition_id_tensor())
        outs = bass2jax._bass_exec_p.bind(
            *operands,
            out_avals=tuple(out_avals),
            in_names=tuple(all_names),
            out_names=tuple(out_names),
            lowering_input_output_aliases=(),
            sim_require_finite=True,
            sim_require_nnan=True,
            nc=nc,
        )
        return tuple(outs)

    devices = jax.devices()[:NCORE]
    mesh = Mesh(np.asarray(devices), ("core",))
    nin = n_params + len(out_names)
    sharded = jax.jit(
        shard_map(
            _body,
            mesh=mesh,
            in_specs=(PartitionSpec("core"),) * nin,
            out_specs=(PartitionSpec("core"),) * len(out_names),
            check_rep=False,
        ),
        keep_unused=True,
    )
    sharding = NamedSharding(mesh, PartitionSpec("core"))
    return sharded, in_names + out_names, out_names, sharding


def _get_state():
    with _lock:
        if "nc" not in _state:
            _state["nc"] = _build_program()
        return _state


def _run_axon(z16, packed):
    import jax

    st = _get_state()
    if "runner" not in st:
        st["runner"] = _axon_runner(st["nc"])
    run, in_names, out_names, sharding = st["runner"]

    key = packed.pop("_key")
    dev = st.get("wdev")
    if dev is None or dev[0] != key:
        # Replicate weights per core by tiling along axis 0 (shard_map
        # hands each core one slice), then park them on device.
        warrs = {
            n: jax.device_put(np.tile(packed[n], (NCORE, 1)), sharding)
            for n in ("w1f", "w2", "w2t", "wout", "bias", "ident")
        }
        zeros = jax.device_put(
            np.zeros((BATCH, DIN), np.float16), sharding
        )
        dev = (key, warrs, zeros)
        st["wdev"] = dev
    _, warrs, zeros = dev

    args = {"z": jax.device_put(z16, sharding), **warrs, "out": zeros}
    outs = run(*[args[n] for n in in_names])
    return np.asarray(outs[out_names.index("out")])


def _run_native(z16, packed):
    from concourse.bass_utils import run_bass_kernel_spmd

    st = _get_state()
    packed = dict(packed)
    packed.pop("_key", None)
    if "tmpdir" not in st:
        st["tmpdir"] = tempfile.mkdtemp(prefix="ghd_neff_")
    in_map_common = {n: packed[n] for n in
                     ("w1f", "w2", "w2t", "wout", "bias", "ident")}
    in_maps = [
        {"z": z16[c * BC:(c + 1) * BC], **in_map_common} for c in range(NCORE)
    ]
    res = run_bass_kernel_spmd(
        st["nc"], in_maps, list(range(NCORE)), tmpdir=st["tmpdir"]
    )
    return np.concatenate([r["out"] for r in res.results], axis=0)


# ------------------------------------------------------------------ kernel ---
def kernel(z, W1, b1, W2, b2, W3, b3, Wf1, bf1, Wf2, bf2):
    z16 = np.asarray(z, np.float32).astype(np.float16)

    st = _get_state()
    key = _weights_key(W1, b1, W2, b2, W3, Wf1, bf1, Wf2, bf2)
    cachedw = st.get("packed")
    if cachedw is None or cachedw["_key"] != key:
        packed = _pack_weights(W1, b1, W2, b2, W3, b3, Wf1, bf1, Wf2, bf2)
        packed["_key"] = key
        st["packed"] = packed
    packed = dict(st["packed"])

    from concourse._compat import axon_active

    if axon_active():
        out16 = _run_axon(z16, packed)
    else:
        out16 = _run_native(z16, packed)
    return out16.astype(np.float32)


# revision 37
# speedup vs baseline: 4.2914x; 1.4116x over previous
"""Generalized Hamiltonian Dynamics — Bass/Tile kernel, data-parallel on 8 NeuronCores.

Strategy (per sharding_hint): shard z along batch (32768 -> 8 x 4096), replicate
the small MLP weights. Per core, feature-major compute in fp16 with fp32 PSUM:

    zT   = PE-transpose(z)                                [64, n]   (dup to 128p)
    h1f  = tanh(W1fT zT + b1f)     W1f=[W1|Wf1]           [2048, n] (16 m-tiles)
    sq1  = h1^2                                           [1024, n]
    h2   = tanh(W2T h1 + b2)                              [1024, n]
    g2n  = (h2^2 - 1) * w3row      (= -g2)                [1024, n]
    un   = W2 g2n                  (= -u)                 [1024, n]
    g1   = (sq1 - 1) * un          (= (1-h1^2)*u)         [1024, n]
    outT = [PW1;Wf2]^T [g1;f1] + bf2                      [64, n]
    out  = PE-transpose(outT)                             [n, 64]

where PW1 = [W1[32:64]; -W1[0:32]] folds the symplectic permutation into the
output projection. L1 (K=64) runs row-packed: two concurrent matmuls in PE
row-groups 0-1 / 2-3 via base_partition 0/64.
"""

import os
import tempfile
import threading

import numpy as np

BATCH, DIN, HID = 32768, 64, 1024
NCORE = 8
BC = BATCH // NCORE      # 4096 rows per core
NT = 512                 # batch tile (matmul free dim)
MT = HID // 128          # 8 m-tiles per hidden layer

_lock = threading.Lock()
_state = {}

# wire/compute dtype for matmul operands: "fp16" or "bf16"
WIRE_DTYPE = os.environ.get("GHD_DTYPE", "fp16")


def _np_wire():
    if WIRE_DTYPE == "bf16":
        import ml_dtypes

        return ml_dtypes.bfloat16
    return np.float16


# ---------------------------------------------------------------- program ----
def _build_program(n_rows=BC, repeat=1, ablate=()):
    from contextlib import ExitStack

    import concourse.bacc as bacc
    import concourse.mybir as mybir
    import concourse.tile as tile
    from concourse.bass import ts

    F16 = (
        mybir.dt.bfloat16 if WIRE_DTYPE == "bf16" else mybir.dt.float16
    )
    F32 = mybir.dt.float32
    TANH = mybir.ActivationFunctionType.Tanh
    IDENT = mybir.ActivationFunctionType.Identity
    MUL = mybir.AluOpType.mult
    SUB = mybir.AluOpType.subtract
    ADD = mybir.AluOpType.add

    nnt = n_rows // NT

    nc = bacc.Bacc(
        "TRN2", target_bir_lowering=False, debug=False, num_devices=NCORE
    )
    z_d = nc.dram_tensor("z", [n_rows, DIN], F16, kind="ExternalInput")
    w1f_d = nc.dram_tensor("w1f", [128, 8 * 128], F16, kind="ExternalInput")
    w2_d = nc.dram_tensor("w2", [128, 64 * 128], F16, kind="ExternalInput")
    w2t_d = nc.dram_tensor("w2t", [128, 64 * 128], F16, kind="ExternalInput")
    wo_d = nc.dram_tensor("wout", [128, 16 * 64], F16, kind="ExternalInput")
    # bias cols: 0:16 b1f | 16:24 b2 | 24:32 w3row | 32 bf2(pad128)
    bias_d = nc.dram_tensor("bias", [128, 33], F32, kind="ExternalInput")
    id_d = nc.dram_tensor("ident", [128, 128], F16, kind="ExternalInput")
    out_d = nc.dram_tensor("out", [n_rows, DIN], F16, kind="ExternalOutput")

    with tile.TileContext(nc) as tc, ExitStack() as ctx:
        wp = ctx.enter_context(tc.tile_pool(name="weights", bufs=1))
        zp = ctx.enter_context(tc.tile_pool(name="zin", bufs=2))
        ztp = ctx.enter_context(tc.tile_pool(name="zt", bufs=2))
        h1p = ctx.enter_context(tc.tile_pool(name="h1f", bufs=2))
        sqp = ctx.enter_context(tc.tile_pool(name="sq1", bufs=2))
        g2p = ctx.enter_context(tc.tile_pool(name="g2", bufs=2))
        g1p = ctx.enter_context(tc.tile_pool(name="g1", bufs=2))
        op = ctx.enter_context(tc.tile_pool(name="osb", bufs=2))
        scr = ctx.enter_context(tc.tile_pool(name="scr", bufs=2))
        ps = ctx.enter_context(tc.tile_pool(name="ps", bufs=1, space="PSUM"))

        w1f_s = wp.tile_from(w1f_d.ap())
        w2_s = wp.tile_from(w2_d.ap())
        w2t_s = wp.tile_from(w2t_d.ap())
        wo_s = wp.tile_from(wo_d.ap())
        bias_s = wp.tile_from(bias_d.ap())
        id_s = wp.tile_from(id_d.ap())

        z_ap = z_d.ap()
        out_ap = out_d.ap()

        def w2blk(w, k, m):
            return w[:, (k * 8 + m) * 128:(k * 8 + m + 1) * 128]

        def stage_T(nt):
            """z load + PE transpose + dup to both partition halves."""
            if "zpre" in ablate:
                # timing ablation: pretend z is already transposed/dup'd
                zt = ztp.tile([128, NT], F16, tag="zt")
                nc.sync.dma_start(
                    out=zt[:],
                    in_=z_ap[0:1024, :].rearrange("(p c) j -> p (c j)", p=128),
                )
                return zt
            zraw = zp.tile([128, 4, 64], F16, tag="zraw")
            src = z_ap[nt * NT:(nt + 1) * NT, :].rearrange(
                "(c p) j -> p c j", p=128
            )
            nc.sync.dma_start(out=zraw[:], in_=src)
            zps = ps.tile([64, NT], F16, tag="psTz", bufs=1)
            for c in range(4):
                nc.tensor.transpose(
                    zps[:, ts(c, 128)], zraw[:, c, :], id_s[:, :]
                )
            zt = ztp.tile([128, NT], F16, tag="zt")
            nc.scalar.copy(zt[0:64, :], zps[:, :])
            nc.vector.tensor_copy(zt[64:128, :], zps[:, :])
            return zt

        def stage_A_alloc():
            h1f = h1p.tile([128, 16, NT], F16, tag="h1f")
            sq1 = sqp.tile([128, 8, NT], F16, tag="sq1")
            return h1f, sq1

        def stage_A_pair(p, zt, h1f, sq1):
            """One L1+F1 row-packed pair (PE row-groups 0-1 / 2-3)."""
            pa = ps.tile([128, NT], F32, tag="psA", bufs=3)
            pb = ps.tile([128, NT], F32, tag="psA", bufs=3)
            nc.tensor.matmul(
                pa[:], lhsT=w1f_s[0:64, ts(p, 128)], rhs=zt[0:64, :],
                start=True, stop=True,
            )
            nc.tensor.matmul(
                pb[:], lhsT=w1f_s[64:128, ts(p, 128)], rhs=zt[64:128, :],
                start=True, stop=True,
            )
            for m, pp in ((2 * p, pa), (2 * p + 1, pb)):
                nc.scalar.activation(
                    h1f[:, m, :], pp[:], TANH, bias=bias_s[:, m:m + 1]
                )
                if m < 8:
                    nc.vector.tensor_tensor(
                        sq1[:, m, :], h1f[:, m, :], h1f[:, m, :], MUL
                    )

        def stage_B(nt, h1f, sq1):
            # ---- L2 forward + g2neg
            g2 = g2p.tile([128, 8, NT], F16, tag="g2")
            for m in range(8):
                pp = ps.tile([128, NT], F32, tag="psBC", bufs=2)
                for k in range(8):
                    nc.tensor.matmul(
                        pp[:], lhsT=w2blk(w2_s, k, m), rhs=h1f[:, k, :],
                        start=(k == 0), stop=(k == 7),
                    )
                h2 = scr.tile([128, NT], F16, tag="h2")
                nc.scalar.activation(
                    h2[:], pp[:], TANH, bias=bias_s[:, 16 + m:17 + m]
                )
                sq2 = scr.tile([128, NT], F16, tag="sq2")
                nc.vector.tensor_tensor(sq2[:], h2[:], h2[:], MUL)
                w3c = bias_s[:, 24 + m:25 + m].to_broadcast((128, NT))
                nc.vector.scalar_tensor_tensor(
                    g2[:, m, :], sq2[:], 1.0, w3c, SUB, MUL
                )
            return g2

        def stage_C_m(m, g2, sq1, g1):
            # ---- L2 backward (one m): un = W2 g2n ; g1 = (sq1-1)*un
            pp = ps.tile([128, NT], F32, tag="psBC", bufs=2)
            for k in range(8):
                nc.tensor.matmul(
                    pp[:], lhsT=w2blk(w2t_s, k, m), rhs=g2[:, k, :],
                    start=(k == 0), stop=(k == 7),
                )
            nc.vector.scalar_tensor_tensor(
                g1[:, m, :], sq1[:, m, :], 1.0, pp[:], SUB, MUL
            )

        def stage_D(nt, g1, h1f):
            if "dserial" in ablate:
                pos = ps.tile([64, NT], F32, tag="psDO", bufs=1)
                for k in range(16):
                    rhs = g1[:, k, :] if k < 8 else h1f[:, k, :]
                    nc.tensor.matmul(
                        pos[:], lhsT=wo_s[:, ts(k, 64)], rhs=rhs,
                        start=(k == 0), stop=(k == 15),
                    )
                osbs = op.tile([64, NT], F16, tag="osb")
                nc.scalar.activation(
                    osbs[:], pos[:], IDENT, bias=bias_s[0:64, 32:33]
                )
                return osbs
            # ---- output projection: outT = [PW1;Wf2]^T [g1;f1] + bf2
            # col-packed: even slots -> PE col-group 0 (psum rows 0:64),
            # odd slots -> col-group 1 (rows 64:128); f1-only tiles first
            # so PE can start before the g1 chain drains.
            po = ps.tile([128, NT], F32, tag="psDO", bufs=1)
            order = list(range(8, 16)) + list(range(8))
            for i, k in enumerate(order):
                rhs = g1[:, k, :] if k < 8 else h1f[:, k, :]
                lo = i % 2 == 0
                nc.tensor.matmul(
                    po[0:64, :] if lo else po[64:128, :],
                    lhsT=wo_s[:, ts(k, 64)], rhs=rhs,
                    start=(i < 2), stop=(i >= 14),
                    tile_position=(0, 0) if lo else (0, 64),
                    skip_group_check=True,
                )
            olo = op.tile([64, NT], F16, tag="olo")
            nc.scalar.activation(
                olo[:], po[0:64, :], IDENT, bias=bias_s[0:64, 32:33]
            )
            osb = op.tile([64, NT], F16, tag="osb")
            nc.vector.tensor_tensor(osb[:], olo[:], po[64:128, :], ADD)
            return osb

        def stage_O(nt, osb):
            if "noout" in ablate:
                # timing ablation: store outT directly, no transpose
                nc.sync.dma_start(
                    out=out_ap[nt * 64:(nt + 1) * 64, 0:64],
                    in_=osb[:, 0:64],
                )
                return
            # ---- transpose back to batch-major and store
            pot = ps.tile([128, 4 * 64], F16, tag="psTo", bufs=1)
            for j in range(4):
                nc.tensor.transpose(
                    pot[:, ts(j, 64)], osb[:, ts(j, 128)], id_s[0:64, 0:64]
                )
            ot2 = op.tile([128, 4, 64], F16, tag="ot2")
            nc.vector.tensor_copy(ot2[:], pot[:])
            dst = out_ap[nt * NT:(nt + 1) * NT, :].rearrange(
                "(c p) j -> p c j", p=128
            )
            nc.sync.dma_start(out=dst, in_=ot2[:])

        # Software pipeline across n-tiles. Emission order (= PE issue
        # order) interleaves independent work so PE never waits on the
        # ACT/DVE drain chains: B(i-1) runs while A(i)'s tanh chain
        # completes; A(i) fills the B(i-1)->C(i-1) gap; D's f1-half
        # fills the C->D gap; O lags one more tile.
        steps = [(rep, nt) for rep in range(repeat) for nt in range(nnt)]
        live = {}

        def emit_AC(key, zt, prev_state):
            """A(i) pairs interleaved with C(i-1) m-blocks: each C block
            (~3.6us of PE) covers the ACT drain of the preceding A pair."""
            h1f, sq1 = stage_A_alloc()
            g1 = None
            if prev_state is not None:
                g1 = g1p.tile([128, 8, NT], F16, tag="g1")
            for j in range(8):
                stage_A_pair(j, zt, h1f, sq1)
                if prev_state is not None:
                    stage_C_m(j, prev_state["g2"], prev_state["sq1"], g1)
            if prev_state is not None:
                prev_state["g1"] = g1
            return h1f, sq1

        for i, key in enumerate(steps):
            zt = stage_T(key[1])
            pk = steps[i - 1] if i >= 1 else None
            ok = steps[i - 2] if i >= 2 else None
            st_p = live.get(pk)
            if st_p is not None:
                st_p["g2"] = stage_B(pk[1], st_p["h1f"], st_p["sq1"])
            h1f, sq1 = emit_AC(key, zt, st_p)
            live[key] = {"h1f": h1f, "sq1": sq1}
            if st_p is not None:
                st_p["osb"] = stage_D(pk[1], st_p["g1"], st_p["h1f"])
            if ok is not None:
                stage_O(ok[1], live.pop(ok)["osb"])
        # epilogue
        last = steps[-1]
        st_p = live[last]
        st_p["g2"] = stage_B(last[1], st_p["h1f"], st_p["sq1"])
        g1 = g1p.tile([128, 8, NT], F16, tag="g1")
        for j in range(8):
            stage_C_m(j, st_p["g2"], st_p["sq1"], g1)
        st_p["g1"] = g1
        st_p["osb"] = stage_D(last[1], st_p["g1"], st_p["h1f"])
        if len(steps) >= 2:
            stage_O(steps[-2][1], live.pop(steps[-2])["osb"])
        stage_O(last[1], live.pop(last)["osb"])

    nc.compile()
    return nc


# ------------------------------------------------------------- host packing --
def _pack_weights(W1, b1, W2, b2, W3, b3, Wf1, bf1, Wf2, bf2):
    f16 = _np_wire()
    W1 = np.asarray(W1, np.float32)
    W2 = np.asarray(W2, np.float32)
    W3 = np.asarray(W3, np.float32)
    Wf1 = np.asarray(Wf1, np.float32)
    Wf2 = np.asarray(Wf2, np.float32)

    W1f = np.concatenate([W1, Wf1], axis=1)            # [64, 2048]
    t = W1f.reshape(64, 8, 2, 128)
    w1f = np.concatenate([t[:, :, 0, :], t[:, :, 1, :]], axis=0)  # [128,8,128]
    w1f = np.ascontiguousarray(w1f.reshape(128, 1024)).astype(f16)

    def pack_kxm(W):                                    # [1024,1024] blocks
        return np.ascontiguousarray(
            W.reshape(8, 128, 8, 128).transpose(1, 0, 2, 3).reshape(128, 8192)
        ).astype(f16)

    w2 = pack_kxm(W2)
    w2t = pack_kxm(np.ascontiguousarray(W2.T))

    # gradT = W1 @ g1T with W1 [DIN=64, HID]; hnnT[r] = gradT[32+r] (r<32)
    # else -gradT[r-32] -> permute/sign W1 rows.
    PW1 = np.concatenate([W1[32:64, :], -W1[0:32, :]], axis=0)  # [64, 1024]
    lhs = np.concatenate(
        [PW1.T.reshape(8, 128, 64), Wf2.reshape(8, 128, 64)], axis=0
    )                                                   # [16,128,64]
    wout = np.ascontiguousarray(
        lhs.transpose(1, 0, 2).reshape(128, 1024)
    ).astype(f16)

    bias = np.zeros((128, 33), np.float32)
    bias[:, 0:16] = np.concatenate(
        [np.asarray(b1, np.float32), np.asarray(bf1, np.float32)]
    ).reshape(16, 128).T
    bias[:, 16:24] = np.asarray(b2, np.float32).reshape(8, 128).T
    bias[:, 24:32] = W3[:, 0].reshape(8, 128).T
    bias[0:64, 32] = np.asarray(bf2, np.float32)

    ident = np.eye(128, dtype=f16)
    return {"w1f": w1f, "w2": w2, "w2t": w2t, "wout": wout,
            "bias": bias, "ident": ident}


def _weights_key(*arrs):
    import hashlib

    h = hashlib.blake2b(digest_size=16)
    for a in arrs:
        a = np.asarray(a)
        h.update(a.shape.__repr__().encode())
        h.update(np.ascontiguousarray(a).view(np.uint8).data)
    return h.hexdigest()


# ----------------------------------------------------------------- runners ---
def _axon_runner(nc):
    """Build a cached jitted executor for the Bass module via PJRT/shard_map."""
    import jax
    import jax.numpy as jnp
    from jax.experimental.shard_map import shard_map
    from jax.sharding import Mesh, NamedSharding, PartitionSpec

    import concourse.mybir as mybir
    from concourse import bass2jax

    bass2jax.install_neuronx_cc_hook()

    partition_name = (
        nc.partition_id_tensor.name if nc.partition_id_tensor else None
    )
    in_names, out_names, out_avals = [], [], []
    for alloc in nc.m.functions[0].allocations:
        if not isinstance(alloc, mybir.MemoryLocationSet):
            continue
        name = alloc.memorylocations[0].name
        if alloc.kind == "ExternalInput":
            if name != partition_name:
                in_names.append(name)
        elif alloc.kind == "ExternalOutput":
            out_names.append(name)
            out_avals.append(
                jax.core.ShapedArray(
                    tuple(alloc.tensor_shape), mybir.dt.np(alloc.dtype)
                )
            )
    n_params = len(in_names)
    all_names = in_names + out_names
    if partition_name is not None:
        all_names.append(partition_name)

    def _body(*args):
        operands = list(args)
        if partition_name is not None:
            operands.append(bass2jax.partition_id_tensor())
        outs = bass2jax._bass_exec_p.bind(
            *operands,
            out_avals=tuple(out_avals),
            in_names=tuple(all_names),
            out_names=tuple(out_names),
            lowering_input_output_aliases=(),
            sim_require_finite=True,
            sim_require_nnan=True,
            nc=nc,
        )
        return tuple(outs)

    devices = jax.devices()[:NCORE]
    mesh = Mesh(np.asarray(devices), ("core",))
    nin = n_params + len(out_names)
    sharded = jax.jit(
        shard_map(
            _body,
            mesh=mesh,
            in_specs=(PartitionSpec("core"),) * nin,
            out_specs=(PartitionSpec("core"),) * len(out_names),
            check_rep=False,
        ),
        keep_unused=True,
    )
    sharding = NamedSharding(mesh, PartitionSpec("core"))
    return sharded, in_names + out_names, out_names, sharding


def _get_state():
    with _lock:
        if "nc" not in _state:
            _state["nc"] = _build_program()
        return _state


def _run_axon(z16, packed):
    import jax

    st = _get_state()
    if "runner" not in st:
        st["runner"] = _axon_runner(st["nc"])
    run, in_names, out_names, sharding = st["runner"]

    key = packed.pop("_key")
    dev = st.get("wdev")
    if dev is None or dev[0] != key:
        # Replicate weights per core by tiling along axis 0 (shard_map
        # hands each core one slice), then park them on device.
        warrs = {
            n: jax.device_put(np.tile(packed[n], (NCORE, 1)), sharding)
            for n in ("w1f", "w2", "w2t", "wout", "bias", "ident")
        }
        zeros = jax.device_put(
            np.zeros((BATCH, DIN), _np_wire()), sharding
        )
        dev = (key, warrs, zeros)
        st["wdev"] = dev
    _, warrs, zeros = dev

    args = {"z": jax.device_put(z16, sharding), **warrs, "out": zeros}
    outs = run(*[args[n] for n in in_names])
    return np.asarray(outs[out_names.index("out")])


def _run_native(z16, packed):
    from concourse.bass_utils import run_bass_kernel_spmd

    st = _get_state()
    packed = dict(packed)
    packed.pop("_key", None)
    if "tmpdir" not in st:
        st["tmpdir"] = tempfile.mkdtemp(prefix="ghd_neff_")
    in_map_common = {n: packed[n] for n in
                     ("w1f", "w2", "w2t", "wout", "bias", "ident")}
    in_maps = [
        {"z": z16[c * BC:(c + 1) * BC], **in_map_common} for c in range(NCORE)
    ]
    res = run_bass_kernel_spmd(
        st["nc"], in_maps, list(range(NCORE)), tmpdir=st["tmpdir"]
    )
    return np.concatenate([r["out"] for r in res.results], axis=0)


# ------------------------------------------------------------------ kernel ---
def kernel(z, W1, b1, W2, b2, W3, b3, Wf1, bf1, Wf2, bf2):
    z16 = np.asarray(z, np.float32).astype(_np_wire())

    st = _get_state()
    key = _weights_key(W1, b1, W2, b2, W3, Wf1, bf1, Wf2, bf2)
    cachedw = st.get("packed")
    if cachedw is None or cachedw["_key"] != key:
        packed = _pack_weights(W1, b1, W2, b2, W3, b3, Wf1, bf1, Wf2, bf2)
        packed["_key"] = key
        st["packed"] = packed
    packed = dict(st["packed"])

    from concourse._compat import axon_active

    if axon_active():
        out16 = _run_axon(z16, packed)
    else:
        out16 = _run_native(z16, packed)
    return out16.astype(np.float32)
